# revision 20
# baseline (speedup 1.0000x reference)
"""Trainium2 Bass kernel for nn_Encoder_55490977464569 (binary-tree GRU encoder).

Strategy
--------
Data-parallel over batch: B=16 -> 2 batch elements per NeuronCore (8 cores),
zero collectives. Each core processes its whole tree (32767 nodes) leaves->root
entirely on-chip: all hidden states live in SBUF (bf16), only `targets` is
streamed in (host pre-transposed to feature-major so no on-device transposes).

All four gate non-linearities are evaluated as tanh on ScalarE
(sigmoid(x) = (tanh(x/2)+1)/2 with the 1/2 folded into host-side weight copies)
so that z|n pre-activations share one PSUM tile and one ACT instruction.

Layout: feature-major [128 features (partitions), node*batch rows (free)].
  - xi GEMMs:  K=32  (lhsT = W_i*.T [32,128] bf16, rhs = x feature-major)
  - hidden GEMMs: K=128 (lhsT = W_h*.T bf16, rhs = hidden bf16)
  - PSUM accumulates xi + hidden GEMMs; ACT applies tanh directly from PSUM.
  - s = rl*hl + rr*hr is folded into the n-GEMM (two strided-rhs matmuls),
    using t2 = (tanh(a/2)+1)*h_child = 2*r*h and W_hn/2.
"""

import sys

if "/opt/trn_rl_repo" not in sys.path:
    sys.path.insert(0, "/opt/trn_rl_repo")
if "/opt/trn_rl_repo/concourse" not in sys.path:
    sys.path.insert(0, "/opt/trn_rl_repo/concourse")

import numpy as np
import ml_dtypes

from concourse import bass, mybir, tile, bacc
from concourse import bass_utils

BF16NP = ml_dtypes.bfloat16
F32 = mybir.dt.float32
BF16 = mybir.dt.bfloat16

N_CORES = 8
DEPTH = 15
HID = 128
IN_DIM = 32
OUT_DIM = 64
BATCH = 16
B_LOCAL = BATCH // N_CORES

T_TILE = 512      # parent rows per tile
H_CHUNK = 1024    # hidden-state chunk width (columns) per SBUF tile

ADD = mybir.AluOpType.add
SUB = mybir.AluOpType.subtract
MULT = mybir.AluOpType.mult
TANH = mybir.ActivationFunctionType.Tanh


def _level_rows(depth, b_local):
    return [2**l * b_local for l in range(depth)]


def _zoff(l, b_local):
    # column offset of level l in xz (heap order: nodes 0..N-1)
    return (2**l - 1) * b_local


def _roff(l, b_local):
    # column offset of level l in xr (levels 0..depth-2, each block 2*R_l wide)
    return (2**l - 1) * 2 * b_local


def build_program(depth=DEPTH, b_local=B_LOCAL, with_mask=False, with_bias=False):
    """Build the Bass program (same SPMD program for every core)."""
    nc = bacc.Bacc("TRN2", target_bir_lowering=False, debug=False,
                   num_devices=1)
    R = _level_rows(depth, b_local)
    total_z = sum(R)
    total_r = sum(2 * R[l] for l in range(depth - 1)) if depth > 1 else 0

    xz_d = nc.dram_tensor("xz", [IN_DIM, total_z], BF16, kind="ExternalInput")
    xr_d = None
    if total_r:
        xr_d = nc.dram_tensor("xr", [IN_DIM, total_r], BF16, kind="ExternalInput")
    # packed x for full-size tiles: 4x 32-row strips -> one [128, 512] DMA
    # feeding 4 concurrently row-tiled K=32 matmuls (tile_position packing).
    pack_levels = [l for l in range(depth - 1) if R[l] >= T_TILE]
    pack_off = {}
    off = 0
    for l in pack_levels:
        pack_off[l] = off
        off += R[l]
    xpack_d = None
    if pack_levels:
        xpack_d = nc.dram_tensor("xpack", [128, off], BF16, kind="ExternalInput")
    leaf_pairs = (R[depth - 1] // T_TILE) // 2 if R[depth - 1] >= 2 * T_TILE else 0
    xleaf_d = None
    if leaf_pairs:
        xleaf_d = nc.dram_tensor("xleaf", [128, leaf_pairs * T_TILE], BF16,
                                 kind="ExternalInput")
    w_irh_d = nc.dram_tensor("w_irh", [IN_DIM, HID], BF16, kind="ExternalInput")
    w_izh_d = nc.dram_tensor("w_izh", [IN_DIM, HID], BF16, kind="ExternalInput")
    w_in_d = nc.dram_tensor("w_in", [IN_DIM, HID], BF16, kind="ExternalInput")
    w_hrh_d = nc.dram_tensor("w_hrh", [HID, HID], BF16, kind="ExternalInput")
    w_hzh_d = nc.dram_tensor("w_hzh", [HID, HID], BF16, kind="ExternalInput")
    w_hnh_d = nc.dram_tensor("w_hnh", [HID, HID], BF16, kind="ExternalInput")
    w_out_d = nc.dram_tensor("w_out", [HID, 2 * OUT_DIM], F32, kind="ExternalInput")
    out_d = nc.dram_tensor("out", [HID, b_local], F32, kind="ExternalOutput")
    if with_bias:
        # per-partition bias columns: [b_r/2 | b_z/2 | b_n | b_zl/2 | b_nl | b_out]
        bias_d = nc.dram_tensor("biases", [HID, 6], F32, kind="ExternalInput")
    if with_mask:
        mask_d = nc.dram_tensor("mask_bc", [HID, total_z], BF16, kind="ExternalInput")

    leaf = depth - 1

    from contextlib import ExitStack
    with tile.TileContext(nc) as tc, ExitStack() as stack:
        consts = stack.enter_context(tc.tile_pool(name="consts", bufs=1))
        hpool = stack.enter_context(tc.tile_pool(name="hpool", bufs=1))
        xpool = stack.enter_context(tc.tile_pool(name="xpool", bufs=6))
        apool = stack.enter_context(tc.tile_pool(name="apool", bufs=8))
        tpool = stack.enter_context(tc.tile_pool(name="tpool", bufs=5))
        pspool = stack.enter_context(tc.tile_pool(name="pspool", bufs=2,
                                                  space="PSUM"))
        opool = stack.enter_context(tc.tile_pool(name="opool", bufs=1))

        def w_tile(name, dram, p, dt):
            t = consts.tile([p, HID], dt, name=name, tag=name)
            nc.sync.dma_start(out=t, in_=dram.ap())
            return t

        w_irh = w_tile("w_irh_sb", w_irh_d, IN_DIM, BF16)
        w_izh = w_tile("w_izh_sb", w_izh_d, IN_DIM, BF16)
        w_in = w_tile("w_in_sb", w_in_d, IN_DIM, BF16)
        w_hrh = w_tile("w_hrh_sb", w_hrh_d, HID, BF16)
        w_hzh = w_tile("w_hzh_sb", w_hzh_d, HID, BF16)
        w_hnh = w_tile("w_hnh_sb", w_hnh_d, HID, BF16)
        w_out = consts.tile([HID, 2 * OUT_DIM], F32, name="w_out_sb", tag="w_out_sb")
        nc.sync.dma_start(out=w_out, in_=w_out_d.ap())
        # strip-combined xi weights for tile_position packing
        wx_sb = consts.tile([128, HID], BF16, name="wx_sb", tag="wx_sb")
        for i, src in enumerate((w_irh_d, w_irh_d, w_izh_d, w_in_d)):
            nc.sync.dma_start(out=wx_sb[32 * i:32 * (i + 1)], in_=src.ap())
        wl_sb = consts.tile([128, HID], BF16, name="wl_sb", tag="wl_sb")
        for i, src in enumerate((w_izh_d, w_in_d, w_izh_d, w_in_d)):
            nc.sync.dma_start(out=wl_sb[32 * i:32 * (i + 1)], in_=src.ap())
        if with_bias:
            bias_sb = consts.tile([HID, 6], F32, name="bias_sb", tag="bias_sb")
            nc.sync.dma_start(out=bias_sb, in_=bias_d.ap())
            b_r, b_z, b_n = bias_sb[:, 0:1], bias_sb[:, 1:2], bias_sb[:, 2:3]
            b_zl, b_nl, b_out = bias_sb[:, 3:4], bias_sb[:, 4:5], bias_sb[:, 5:6]

        # PE warm-up: ~8.5us of dense back-to-back matmuls un-throttles the
        # HAM clock gate (1.2 -> 2.4 GHz); all later matmuls run warm as long
        # as the PE never idles a full ~3.4us window.
        warm_x = consts.tile([128, 512], BF16, name="warm_x", tag="warm_x")
        nc.gpsimd.memset(warm_x, 0.0)
        ps_warm = pspool.tile([HID, 512], F32, name="ps_warm", tag="ps_rr")
        for _ in range(20):
            nc.tensor.matmul(ps_warm, wx_sb, warm_x, start=True, stop=True)

        # hidden-state tiles: h[l][c] is chunk c of level l (bf16)
        h_tiles = []
        for l in range(depth):
            cw = min(H_CHUNK, R[l])
            n_chunks = (R[l] + cw - 1) // cw
            h_tiles.append([
                hpool.tile([HID, cw], BF16, name=f"h_{l}_{c}", tag=f"h_{l}_{c}")
                for c in range(n_chunks)
            ])

        def mask_mul_inplace(view, lvl, col0, width):
            m_sb = xpool.tile([HID, width], BF16, name="m_sb", tag="m_sb")
            nc.sync.dma_start(
                out=m_sb, in_=mask_d.ap()[:, _zoff(lvl, b_local) + col0:
                                          _zoff(lvl, b_local) + col0 + width])
            nc.vector.tensor_mul(view, view, m_sb)

        # ---------------- leaf level ----------------
        def leaf_tail(zn, Tl, t0, k):
            """activation + h for one leaf tile, given its [z|n] psum done."""
            zz = tpool.tile([HID, Tl], BF16, name="zz_leaf", tag="cs")
            nc.vector.tensor_scalar(zz, zn[:, 0:Tl], -0.5, 0.5, op0=MULT, op1=ADD)
            cidx, coff = t0 // H_CHUNK, t0 % H_CHUNK
            hview = h_tiles[leaf][cidx][:, coff:coff + Tl]
            # gpsimd is otherwise idle; the final leaf multiply runs there to
            # keep VectorE free for the interior-level pipeline
            nc.gpsimd.tensor_mul(hview, zz, zn[:, Tl:2 * Tl])
            if with_mask:
                mask_mul_inplace(hview, leaf, t0, Tl)

        Tl = min(T_TILE, R[leaf])
        n_leaf_tiles = R[leaf] // Tl
        for j in range(leaf_pairs):
            # two leaf tiles (2j, 2j+1) share one [128, T] packed x DMA and
            # four concurrently row-tiled K=32 matmuls.
            xp = xpool.tile([128, Tl], BF16, name="xp_leaf", tag="xp")
            nc.sync.dma_start(out=xp, in_=xleaf_d.ap()[:, j * Tl:(j + 1) * Tl])
            pss = []
            for u in range(2):
                ps = pspool.tile([HID, 2 * Tl], F32, name="ps_leaf",
                                 tag="ps_rr")
                for i in range(2):
                    s = 2 * u + i
                    nc.tensor.matmul(ps[:, i * Tl:(i + 1) * Tl],
                                     wl_sb[32 * s:32 * (s + 1)],
                                     xp[32 * s:32 * (s + 1)],
                                     start=True, stop=True,
                                     tile_position=(32 * s, 0))
                pss.append(ps)
            for u in range(2):
                k = 2 * j + u
                zn = apool.tile([HID, 2 * Tl], BF16, name="zn_leaf", tag="act")
                if with_bias:
                    nc.scalar.activation(zn[:, 0:Tl], pss[u][:, 0:Tl], TANH,
                                         bias=b_zl)
                    nc.scalar.activation(zn[:, Tl:2 * Tl], pss[u][:, Tl:2 * Tl],
                                         TANH, bias=b_nl)
                else:
                    nc.scalar.activation(zn, pss[u], TANH)
                leaf_tail(zn, Tl, k * Tl, k)

        for k in range(2 * leaf_pairs, n_leaf_tiles):
            t0 = k * Tl
            xz_sb = xpool.tile([IN_DIM, Tl], BF16, name="xz_sb", tag="xz")
            nc.sync.dma_start(out=xz_sb,
                              in_=xz_d.ap()[:, _zoff(leaf, b_local) + t0:
                                            _zoff(leaf, b_local) + t0 + Tl])
            ps = pspool.tile([HID, 2 * Tl], F32, name="ps_leaf",
                             tag="ps_rr")
            nc.tensor.matmul(ps[:, 0:Tl], w_izh, xz_sb, start=True, stop=True)
            nc.tensor.matmul(ps[:, Tl:2 * Tl], w_in, xz_sb, start=True, stop=True)
            zn = apool.tile([HID, 2 * Tl], BF16, name="zn_leaf", tag="act")
            if with_bias:
                nc.scalar.activation(zn[:, 0:Tl], ps[:, 0:Tl], TANH, bias=b_zl)
                nc.scalar.activation(zn[:, Tl:2 * Tl], ps[:, Tl:2 * Tl], TANH,
                                     bias=b_nl)
            else:
                nc.scalar.activation(zn, ps, TANH)
            leaf_tail(zn, Tl, t0, k)

        # ---------------- interior levels ----------------
        for l in range(depth - 2, -1, -1):
            T = min(T_TILE, R[l])
            C_child = min(H_CHUNK, R[l + 1])
            C_own = min(H_CHUNK, R[l])
            packed = l in pack_off
            for k in range(R[l] // T):
                t0 = k * T
                cw = 2 * T
                cidx, coff = (2 * t0) // C_child, (2 * t0) % C_child
                child = h_tiles[l + 1][cidx][:, coff:coff + cw]

                ps_rr = pspool.tile([HID, cw], F32, name="ps_rr", tag="ps_rr")
                if packed:
                    ps_z = pspool.tile([HID, T], F32, name="ps_z", tag="ps_z")
                    ps_n = pspool.tile([HID, T], F32, name="ps_n", tag="ps_n")
                    # one [128, T] DMA; 4 concurrently row-tiled K=32 matmuls
                    # (xi_r lo, xi_r hi, xi_z, xi_n)
                    xp = xpool.tile([128, T], BF16, name="xp_sb", tag="xp")
                    nc.sync.dma_start(out=xp,
                                      in_=xpack_d.ap()[:, pack_off[l] + t0:
                                                       pack_off[l] + t0 + T])
                    for s, dst in enumerate((ps_rr[:, 0:T], ps_rr[:, T:2 * T],
                                             ps_z, ps_n)):
                        nc.tensor.matmul(dst, wx_sb[32 * s:32 * (s + 1)],
                                         xp[32 * s:32 * (s + 1)],
                                         start=True, stop=False,
                                         tile_position=(32 * s, 0))
                else:
                    ps_zn = pspool.tile([HID, 2 * T], F32, name="ps_zn",
                                        tag="ps_z")
                    xr_sb = xpool.tile([IN_DIM, cw], BF16, name="xr_sb", tag="xr")
                    nc.sync.dma_start(out=xr_sb,
                                      in_=xr_d.ap()[:, _roff(l, b_local) + 2 * t0:
                                                    _roff(l, b_local) + 2 * t0 + cw])
                    xz_sb = xpool.tile([IN_DIM, T], BF16, name="xz_sb", tag="xz")
                    nc.sync.dma_start(out=xz_sb,
                                      in_=xz_d.ap()[:, _zoff(l, b_local) + t0:
                                                    _zoff(l, b_local) + t0 + T])
                    for i in range((cw + 511) // 512):
                        sl = slice(i * 512, min((i + 1) * 512, cw))
                        nc.tensor.matmul(ps_rr[:, sl], w_irh, xr_sb[:, sl],
                                         start=True, stop=False)

                # rr psum: (xi_r + h_child @ W_hr)/2, child-row order
                for i in range((cw + 511) // 512):
                    sl = slice(i * 512, min((i + 1) * 512, cw))
                    nc.tensor.matmul(ps_rr[:, sl], w_hrh, child[:, sl],
                                     start=False, stop=True)
                r_sb = apool.tile([HID, cw], BF16, name="r_sb", tag="act")
                if with_bias:
                    nc.scalar.activation(r_sb, ps_rr, TANH, bias=b_r)
                else:
                    nc.scalar.activation(r_sb, ps_rr, TANH)

                # child sum cs = hl + hr
                cs_sb = tpool.tile([HID, T], BF16, name="cs_sb", tag="cs")
                ch3 = child.rearrange("p (g f) -> p g f", f=4)
                cs3 = cs_sb.rearrange("p (g f) -> p g f", f=2)
                nc.vector.tensor_add(cs3, ch3[:, :, 0:2], ch3[:, :, 2:4])

                # t2 = (tau_r + 1) * h_child = 2 * r * h_child
                t2_sb = tpool.tile([HID, cw], BF16, name="t2_sb", tag="t2")
                nc.vector.scalar_tensor_tensor(t2_sb, r_sb, 1.0, child,
                                               op0=ADD, op1=MULT)

                # z psum: (xi_z + cs@W_hz)/2 ; n psum: xi_n + s@W_hn
                # (legacy small-T path keeps both in one tile/zero-region, so
                # each accumulation group completes before the next starts)
                t23 = t2_sb.rearrange("p (g f) -> p g f", f=4)
                if packed:
                    nc.tensor.matmul(ps_z, w_hzh, cs_sb, start=False, stop=True)
                    tz_sb = apool.tile([HID, T], BF16, name="tz_sb", tag="act_s")
                    nc.scalar.activation(tz_sb, ps_z, TANH,
                                         **(dict(bias=b_z) if with_bias else {}))
                    nc.tensor.matmul(ps_n, w_hnh, t23[:, :, 0:2],
                                     start=False, stop=False)
                    nc.tensor.matmul(ps_n, w_hnh, t23[:, :, 2:4],
                                     start=False, stop=True)
                    n_sb = apool.tile([HID, T], BF16, name="n_sb", tag="act_s")
                    nc.scalar.activation(n_sb, ps_n, TANH,
                                         **(dict(bias=b_n) if with_bias else {}))
                else:
                    nc.tensor.matmul(ps_zn[:, 0:T], w_izh, xz_sb,
                                     start=True, stop=False)
                    nc.tensor.matmul(ps_zn[:, 0:T], w_hzh, cs_sb,
                                     start=False, stop=True)
                    nc.tensor.matmul(ps_zn[:, T:2 * T], w_in, xz_sb,
                                     start=True, stop=False)
                    nc.tensor.matmul(ps_zn[:, T:2 * T], w_hnh, t23[:, :, 0:2],
                                     start=False, stop=False)
                    nc.tensor.matmul(ps_zn[:, T:2 * T], w_hnh, t23[:, :, 2:4],
                                     start=False, stop=True)
                    zn_sb = apool.tile([HID, 2 * T], BF16, name="zn_sb",
                                       tag="act")
                    if with_bias:
                        nc.scalar.activation(zn_sb[:, 0:T], ps_zn[:, 0:T], TANH,
                                             bias=b_z)
                        nc.scalar.activation(zn_sb[:, T:2 * T],
                                             ps_zn[:, T:2 * T], TANH, bias=b_n)
                    else:
                        nc.scalar.activation(zn_sb, ps_zn, TANH)
                    tz_sb = zn_sb[:, 0:T]
                    n_sb = zn_sb[:, T:2 * T]

                # h = n + z*(cs - n),  z = (tau_z + 1)/2
                d_sb = tpool.tile([HID, T], BF16, name="d_sb", tag="d")
                nc.gpsimd.tensor_sub(d_sb, cs_sb, n_sb)
                z_sb = tpool.tile([HID, T], BF16, name="z_sb", tag="v")
                nc.vector.tensor_scalar(z_sb, tz_sb, 0.5, 0.5,
                                        op0=MULT, op1=ADD)
                zd_sb = tpool.tile([HID, T], BF16, name="zd_sb", tag="zd")
                nc.vector.tensor_mul(zd_sb, z_sb, d_sb)
                hidx, hoff = t0 // C_own, t0 % C_own
                hview = h_tiles[l][hidx][:, hoff:hoff + T]
                nc.vector.tensor_add(hview, zd_sb, n_sb)
                if with_mask:
                    mask_mul_inplace(hview, l, t0, T)

        # ---------------- output head ----------------
        h0f = tpool.tile([HID, b_local], F32, name="h0f", tag="h0f")
        nc.vector.tensor_copy(h0f, h_tiles[0][0])
        ps_out = pspool.tile([HID, b_local], F32, name="ps_out", tag="ps_z")
        nc.tensor.matmul(ps_out, w_out, h0f, start=True, stop=True)
        out_sb = opool.tile([HID, b_local], F32, name="out_sb", tag="out_sb")
        if with_bias:
            nc.scalar.activation(out_sb, ps_out,
                                 mybir.ActivationFunctionType.Identity,
                                 bias=b_out)
        else:
            nc.scalar.copy(out_sb, ps_out)
        nc.sync.dma_start(out=out_d.ap(), in_=out_sb)

    nc.compile()
    return nc


def host_prep(inputs, depth=DEPTH, b_local=B_LOCAL, n_cores=N_CORES,
              with_mask=False, with_bias=False):
    """Build per-core input maps from the full problem inputs."""
    t = np.ascontiguousarray(np.asarray(inputs["targets"], np.float32))
    N = t.shape[0]
    assert N == 2**depth - 1 and t.shape[2] == IN_DIM
    R = _level_rows(depth, b_local)

    # feature-major, bf16: [32, N, B]
    xt = np.ascontiguousarray(t.transpose(2, 0, 1)).astype(BF16NP)

    def half_t(w):  # (W/2).T as bf16 [in, out]
        return np.ascontiguousarray(np.asarray(w, np.float32).T * 0.5).astype(BF16NP)

    def plain_t(w):
        return np.ascontiguousarray(np.asarray(w, np.float32).T).astype(BF16NP)

    w_irh = half_t(inputs["W_ir"])
    w_izh = half_t(inputs["W_iz"])
    w_in = plain_t(inputs["W_in"])
    w_hrh = half_t(inputs["W_hr"])
    w_hzh = half_t(inputs["W_hz"])
    w_hnh = half_t(inputs["W_hn"])
    w_out = np.ascontiguousarray(
        np.concatenate([np.asarray(inputs["W_mu"], np.float32),
                        np.asarray(inputs["W_lv"], np.float32)], axis=0).T)

    shared = dict(w_irh=w_irh, w_izh=w_izh, w_in=w_in, w_hrh=w_hrh,
                  w_hzh=w_hzh, w_hnh=w_hnh, w_out=w_out)
    if with_bias:
        b = {k: np.asarray(inputs[k], np.float32) for k in
             ("b_ir", "b_hr", "b_iz", "b_hz", "b_in", "b_hn", "b_mu", "b_lv")}
        bias = np.zeros((HID, 6), np.float32)
        bias[:, 0] = 0.5 * (b["b_ir"] + b["b_hr"])
        bias[:, 1] = 0.5 * (b["b_iz"] + b["b_hz"])
        bias[:, 2] = b["b_in"] + b["b_hn"]
        # leaves: child_sum = s = 0, but b_hz / b_hn still apply in the reference
        bias[:, 3] = 0.5 * (b["b_iz"] + b["b_hz"])
        bias[:, 4] = b["b_in"] + b["b_hn"]
        bias[:128, 5] = np.concatenate([b["b_mu"], b["b_lv"]])
        shared["biases"] = bias

    in_maps = []
    for c in range(n_cores):
        b0 = c * b_local
        xz = np.ascontiguousarray(
            xt[:, :, b0:b0 + b_local].reshape(IN_DIM, N * b_local))
        blocks = []
        for l in range(depth - 1):
            blk = xz[:, _zoff(l, b_local):_zoff(l, b_local) + R[l]]
            rep = np.repeat(blk.reshape(IN_DIM, -1, 1, 2), 2, axis=2)
            blocks.append(rep.reshape(IN_DIM, 2 * R[l]))
        m = dict(shared)
        m["xz"] = xz
        xr = np.concatenate(blocks, axis=1) if blocks else None
        if xr is not None:
            m["xr"] = np.ascontiguousarray(xr)
        # packed [128, T] blocks for tile_position-packed xi matmuls
        pack_levels = [l for l in range(depth - 1) if R[l] >= T_TILE]
        if pack_levels:
            pblocks = []
            for l in pack_levels:
                for k in range(R[l] // T_TILE):
                    t0 = k * T_TILE
                    rblk = xr[:, _roff(l, b_local) + 2 * t0:
                              _roff(l, b_local) + 2 * t0 + 2 * T_TILE]
                    zblk = xz[:, _zoff(l, b_local) + t0:
                              _zoff(l, b_local) + t0 + T_TILE]
                    pblocks.append(np.concatenate(
                        [rblk[:, :T_TILE], rblk[:, T_TILE:], zblk, zblk], axis=0))
            m["xpack"] = np.ascontiguousarray(np.concatenate(pblocks, axis=1))
        leaf = depth - 1
        leaf_pairs = (R[leaf] // T_TILE) // 2 if R[leaf] >= 2 * T_TILE else 0
        if leaf_pairs:
            lblocks = []
            for j in range(leaf_pairs):
                za = xz[:, _zoff(leaf, b_local) + 2 * j * T_TILE:
                        _zoff(leaf, b_local) + (2 * j + 1) * T_TILE]
                zb = xz[:, _zoff(leaf, b_local) + (2 * j + 1) * T_TILE:
                        _zoff(leaf, b_local) + (2 * j + 2) * T_TILE]
                lblocks.append(np.concatenate([za, za, zb, zb], axis=0))
            m["xleaf"] = np.ascontiguousarray(np.concatenate(lblocks, axis=1))
        if with_mask:
            mk = np.asarray(inputs["mask"], np.float32)[:, b0:b0 + b_local]
            m["mask_bc"] = np.ascontiguousarray(
                np.broadcast_to(mk.reshape(1, N * b_local),
                                (HID, N * b_local))).astype(BF16NP)
        in_maps.append(m)
    return in_maps


_PROGRAM_CACHE = {}


def _get_program(with_mask, with_bias):
    key = (with_mask, with_bias)
    if key not in _PROGRAM_CACHE:
        _PROGRAM_CACHE[key] = build_program(with_mask=with_mask,
                                            with_bias=with_bias)
    return _PROGRAM_CACHE[key]


def run_on_device(inputs, trace=False, **trace_kw):
    with_mask = not np.all(np.asarray(inputs["mask"]) == 1.0)
    with_bias = any(
        np.any(np.asarray(inputs[k]) != 0.0)
        for k in ("b_ir", "b_hr", "b_iz", "b_hz", "b_in", "b_hn", "b_mu", "b_lv"))
    nc = _get_program(with_mask, with_bias)
    in_maps = host_prep(inputs, with_mask=with_mask, with_bias=with_bias)
    res = bass_utils.run_bass_kernel_spmd(
        nc, in_maps, core_ids=list(range(N_CORES)), trace=trace, **trace_kw)
    mu = np.zeros((BATCH, OUT_DIM), np.float32)
    lv = np.zeros((BATCH, OUT_DIM), np.float32)
    for c in range(N_CORES):
        o = res.results[c]["out"]  # [128, b_local]
        mu[c * B_LOCAL:(c + 1) * B_LOCAL] = o[:OUT_DIM].T
        lv[c * B_LOCAL:(c + 1) * B_LOCAL] = o[OUT_DIM:].T
    return (mu, lv), res


def kernel(**inputs):
    (mu, lv), _ = run_on_device(inputs)
    return mu, lv
